# revision 1
# baseline (speedup 1.0000x reference)
"""Causal GQA attention block (QK L2-norm + RoPE) for 8 trn2 NeuronCores.

Sharding: tensor-parallel over head-halves (2) x data-parallel over batch (4).
Core c handles batch c//2 and heads [h*8, h*8+8) with h = c%2 (kv heads
[h*2, h*2+2)).  Each core computes its partial output-projection
out_part^T = w_o[:, cols].T-contraction; the host sums the two partials per
batch and transposes back.

Layouts on device (chosen so every reduction is a TensorE contraction over
partitions and every softmax op is a cheap per-partition ACT/DVE op):
  - x^T, Q^T, K^T: [feature(d), token(t)]  (d on partitions)
  - V:             [token, feature]        (t on partitions)
  - scores^T:      [k, q] so AV needs no transpose; softmax row-sums come
                   from an all-ones matmul; the max-subtraction is skipped
                   because QK-norm bounds scores to +-0.0884.
  - RoPE rotate-half is a signed 128x128 permutation matmul plus two
    elementwise multiplies with host-provided cos/sin tables.
All matmuls run in float32r (fast PE mode).
"""

import numpy as np

import concourse.mybir as mybir
import concourse.tile as tile
from concourse import bacc
from concourse import bass2jax

F32 = mybir.dt.float32
F32R = mybir.dt.float32r
AF = mybir.ActivationFunctionType

P = 128
B, T, D = 4, 2048, 2048
N_HEADS, HEAD_DIM, N_KV = 16, 128, 4
Q_DIM = N_HEADS * HEAD_DIM          # 2048
KV_DIM = N_KV * HEAD_DIM            # 512
H_Q = 8                             # q heads per core
H_KV = 2                            # kv heads per core
EQ = H_Q * HEAD_DIM                 # 1024 q features per core
EKV = H_KV * HEAD_DIM               # 256
SCALE = 0.08838834764831845
THETA = 10000.0

KSUB = D // P                       # 16 contraction subtiles
N_CORES = 8
TT_HALF = T // 2                    # 1024, phase-1 token half
NT512 = T // 512                    # 4 512-token tiles
NTB = T // P                        # 16 128-token blocks


def _build_module():
    nc = bacc.Bacc("TRN2", target_bir_lowering=False, debug=False)

    xt = nc.dram_tensor("xt", [D, T], F32R, kind="ExternalInput")
    wq = nc.dram_tensor("wq", [H_Q, P, KSUB, P], F32R, kind="ExternalInput")
    wk = nc.dram_tensor("wk", [P, KSUB, EKV], F32R, kind="ExternalInput")
    wv = nc.dram_tensor("wv", [P, KSUB, EKV], F32R, kind="ExternalInput")
    wo = nc.dram_tensor("wo", [P, H_Q, D], F32R, kind="ExternalInput")
    cos_t = nc.dram_tensor("cos_t", [P, T], F32R, kind="ExternalInput")
    sin_t = nc.dram_tensor("sin_t", [P, T], F32R, kind="ExternalInput")
    ones_m = nc.dram_tensor("ones_m", [P, P], F32R, kind="ExternalInput")
    pswap = nc.dram_tensor("pswap", [P, P], F32R, kind="ExternalInput")
    out_t = nc.dram_tensor("out_t", [D, T], F32, kind="ExternalOutput")

    with tile.TileContext(nc) as tc:
        with (
            tc.tile_pool(name="persist", bufs=1) as persist,
            tc.tile_pool(name="kv_persist", bufs=1) as kvp,
            tc.tile_pool(name="qdram", bufs=1, space="DRAM") as qdram,
            # attention-critical sbuf pools, pre-allocated so their
            # addresses never overlap phase-1 pools
            tc.tile_pool(name="qstream", bufs=3) as qstream,
            tc.tile_pool(name="att_sb", bufs=5) as att_sb,
        ):
            ones_sb = persist.tile([P, P], F32R)
            psw_sb = persist.tile([P, P], F32R)
            nc.sync.dma_start(ones_sb[:], ones_m.ap())
            nc.sync.dma_start(psw_sb[:], pswap.ap())
            k_sb = kvp.tile([P, H_KV, T], F32R)       # roped+normed K^T slabs
            v_sb = kvp.tile([P, NTB, EKV], F32R)      # V in [t, e] layout
            q_scr = [
                [
                    qdram.tile([P, 512], F32R, name=f"qscr_{h}_{t}")
                    for t in range(NT512)
                ]
                for h in range(H_Q)
            ]

            # ---------------- phase 1: qkv proj + L2 norm + rope ----------
            with (
                tc.tile_pool(name="xres", bufs=1) as xres,
                tc.tile_pool(name="wstream", bufs=2) as wstream,
                tc.tile_pool(name="wvres", bufs=1) as wvres,
                tc.tile_pool(name="p1tmp", bufs=2) as p1tmp,
                tc.tile_pool(name="p1out", bufs=2) as p1out,
                tc.tile_pool(name="trig", bufs=1) as trig,
                tc.tile_pool(name="pp", bufs=2, space="PSUM") as pp,
                tc.tile_pool(name="pssq", bufs=2, space="PSUM") as pssq,
                tc.tile_pool(name="psw", bufs=2, space="PSUM") as psw,
                tc.tile_pool(name="pv", bufs=2, space="PSUM") as pv,
            ):
                cos_sb = trig.tile([P, T], F32R)
                sin_sb = trig.tile([P, T], F32R)
                wv_sb = wvres.tile([P, KSUB, EKV], F32R)
                # K weights resident up front: the first projections are K,
                # and their lhsT must not queue behind the x-tile DMAs
                wk_sb = wvres.tile([P, KSUB, EKV], F32R, name="wk_sb")
                nc.sync.dma_start(wk_sb[:], wk.ap())
                for th in range(2):
                    t0 = th * TT_HALF
                    x_sb = xres.tile([P, KSUB, TT_HALF], F32R, tag="x")
                    xr = xt.ap()[:, t0 : t0 + TT_HALF].rearrange(
                        "(ks p) t -> p ks t", p=P
                    )
                    for ks in range(KSUB):
                        nc.sync.dma_start(x_sb[:, ks], xr[:, ks])
                    if th == 0:
                        # needed only from the first norm/rope (~35us in) and
                        # V projections; keep them behind the x stream
                        nc.sync.dma_start(cos_sb[:], cos_t.ap())
                        nc.sync.dma_start(sin_sb[:], sin_t.ap())
                        nc.sync.dma_start(wv_sb[:], wv.ap())

                    def proj_norm_rope(es):
                        """project feature block es, normalize, rope"""
                        if es < H_Q:
                            w_sb = wstream.tile([P, KSUB, P], F32R, tag="w")
                            nc.sync.dma_start(w_sb[:], wq.ap()[es])
                            w_use = w_sb[:]
                        else:
                            e0 = (es - H_Q) * P
                            w_use = wk_sb[:, :, e0 : e0 + P]
                        for tt in range(2):
                            tg = t0 + tt * 512
                            sl = slice(tt * 512, (tt + 1) * 512)
                            raw_ps = pp.tile([P, 512], F32, tag="raw")
                            for ks in range(KSUB):
                                nc.tensor.matmul(
                                    raw_ps[:],
                                    w_use[:, ks],
                                    x_sb[:, ks, sl],
                                    start=(ks == 0),
                                    stop=(ks == KSUB - 1),
                                )
                            sq = p1tmp.tile([P, 512], F32R, tag="t1")
                            nc.scalar.activation(sq[:], raw_ps[:], AF.Square)
                            ssq_ps = pssq.tile([P, 512], F32, tag="ssq")
                            nc.tensor.matmul(
                                ssq_ps[:], ones_sb[:], sq[:], start=True, stop=True
                            )
                            s_sb = p1tmp.tile([P, 512], F32, tag="t2")
                            nc.scalar.activation(s_sb[:], ssq_ps[:], AF.Sqrt)
                            r_sb = p1tmp.tile([P, 512], F32, tag="t3")
                            nc.vector.reciprocal_approx_fast(r_sb[:], s_sb[:])
                            qn = p1tmp.tile([P, 512], F32R, tag="t4")
                            nc.vector.tensor_mul(qn[:], raw_ps[:], r_sb[:])
                            ys = p1tmp.tile([P, 512], F32R, tag="t1")
                            nc.vector.tensor_mul(
                                ys[:], qn[:], sin_sb[:, tg : tg + 512]
                            )
                            sw_ps = psw.tile([P, 512], F32, tag="sw")
                            nc.tensor.matmul(
                                sw_ps[:], psw_sb[:], ys[:], start=True, stop=True
                            )
                            qc = p1tmp.tile([P, 512], F32, tag="t2")
                            nc.vector.tensor_mul(
                                qc[:], qn[:], cos_sb[:, tg : tg + 512]
                            )
                            if es < H_Q:
                                rope = p1out.tile([P, 512], F32R, tag="rope")
                                nc.vector.tensor_add(rope[:], sw_ps[:], qc[:])
                                nc.sync.dma_start(
                                    q_scr[es][tg // 512][:], rope[:]
                                )
                            else:
                                nc.vector.tensor_add(
                                    k_sb[:, es - H_Q, tg : tg + 512],
                                    sw_ps[:],
                                    qc[:],
                                )

                    # K first so attention can start earliest, then Q, then V
                    for es in (H_Q, H_Q + 1):
                        proj_norm_rope(es)
                    for es in range(H_Q):
                        proj_norm_rope(es)
                    for tb in range(TT_HALF // P):
                        tbg = th * (TT_HALF // P) + tb
                        v_ps = pv.tile([P, EKV], F32, tag="vp")
                        for ks in range(KSUB):
                            nc.tensor.matmul(
                                v_ps[:],
                                x_sb[:, ks, tb * P : (tb + 1) * P],
                                wv_sb[:, ks],
                                start=(ks == 0),
                                stop=(ks == KSUB - 1),
                            )
                        nc.scalar.copy(v_sb[:, tbg], v_ps[:])

            # ------- phase 2: attention + output projection per q-tile ----
            with (
                tc.tile_pool(name="wores", bufs=1) as wores,
                tc.tile_pool(name="p2tmp", bufs=2) as p2tmp,
                tc.tile_pool(name="oall", bufs=2) as oall,
                tc.tile_pool(name="fout", bufs=3) as fout,
                tc.tile_pool(name="psc", bufs=2, space="PSUM") as psc,
                tc.tile_pool(name="pav", bufs=1, space="PSUM") as pav,
                tc.tile_pool(name="psum2", bufs=1, space="PSUM") as psum2,
                tc.tile_pool(name="pf", bufs=2, space="PSUM") as pf,
            ):
                # w_o via the (idle) gpsimd DMA queue, split per slab, so it
                # never head-of-line-blocks the sync queue's q-tile loads
                wo_sb = wores.tile([P, H_Q, D], F32R)
                for ei in range(H_Q):
                    nc.gpsimd.dma_start(wo_sb[:, ei], wo.ap()[:, ei])
                for qt in range(NT512):
                    q0 = qt * 512
                    nkb = (qt + 1) * 4
                    o_all = oall.tile([P, H_Q, 512], F32R, tag="oa")
                    for hd in range(H_Q):
                        kvi = hd // 4
                        q_t = qstream.tile([P, 512], F32R, tag="q")
                        nc.sync.dma_start(q_t[:], q_scr[hd][qt][:])
                        atts = []

                        def diag_off(kb):
                            # left columns of a diagonal block that are fully
                            # masked; only skip when >=256 wide remains so
                            # fp32r keeps its fast mode
                            off = kb * P - q0
                            return off if off in (P, 2 * P) else 0

                        for kb0 in range(0, nkb, 2):
                            npair = min(2, nkb - kb0)
                            sc_ps = psc.tile([P, 1024], F32, tag="sc")
                            att = att_sb.tile([P, 1024], F32R, tag="att")
                            for j in range(npair):
                                kb = kb0 + j
                                off = diag_off(kb)
                                nc.tensor.matmul(
                                    sc_ps[:, j * 512 + off : (j + 1) * 512],
                                    k_sb[:, kvi, kb * P : (kb + 1) * P],
                                    q_t[:, off:],
                                    start=True,
                                    stop=True,
                                )
                            offs = [diag_off(kb0 + j) for j in range(npair)]
                            if not any(offs):
                                nc.scalar.activation(
                                    att[:, : npair * 512],
                                    sc_ps[:, : npair * 512],
                                    AF.Exp,
                                    scale=SCALE,
                                )
                            else:
                                for j in range(npair):
                                    sl = slice(j * 512 + offs[j], (j + 1) * 512)
                                    nc.scalar.activation(
                                        att[:, sl], sc_ps[:, sl], AF.Exp,
                                        scale=SCALE,
                                    )
                            for j in range(npair):
                                kb = kb0 + j
                                off = offs[j]
                                # zero future positions on diagonal blocks
                                if q0 < (kb + 1) * P and kb * P < q0 + 512:
                                    sl = slice(j * 512 + off, (j + 1) * 512)
                                    nc.gpsimd.affine_select(
                                        out=att[:, sl],
                                        in_=att[:, sl],
                                        compare_op=mybir.AluOpType.is_ge,
                                        fill=0.0,
                                        base=q0 + off - kb * P,
                                        pattern=[[1, 512 - off]],
                                        channel_multiplier=-1,
                                    )
                                atts.append((kb, att[:, j * 512 : (j + 1) * 512]))
                        o_ps = pav.tile([P, 512], F32, tag="av")
                        for kb, a_slice in atts:
                            off = diag_off(kb)
                            nc.tensor.matmul(
                                o_ps[:, off:],
                                v_sb[:, kb, kvi * HEAD_DIM : (kvi + 1) * HEAD_DIM],
                                a_slice[:, off:],
                                start=(kb == 0),
                                stop=(kb == nkb - 1),
                            )
                        s_ps = psum2.tile([P, 512], F32, tag="sum")
                        for kb, a_slice in atts:
                            off = diag_off(kb)
                            nc.tensor.matmul(
                                s_ps[:, off:],
                                ones_sb[:],
                                a_slice[:, off:],
                                start=(kb == 0),
                                stop=(kb == nkb - 1),
                            )
                        rs = p2tmp.tile([P, 512], F32, tag="rs")
                        nc.vector.reciprocal_approx_fast(rs[:], s_ps[:])
                        nc.vector.tensor_mul(o_all[:, hd], o_ps[:], rs[:])
                    for eo in range(D // P):
                        f_ps = pf.tile([P, 512], F32, tag="f")
                        for ei in range(H_Q):
                            nc.tensor.matmul(
                                f_ps[:],
                                wo_sb[:, ei, eo * P : (eo + 1) * P],
                                o_all[:, ei],
                                start=(ei == 0),
                                stop=(ei == H_Q - 1),
                            )
                        f_sb = fout.tile([P, 512], F32, tag="fo")
                        nc.scalar.copy(f_sb[:], f_ps[:])
                        nc.sync.dma_start(
                            out_t.ap()[eo * P : (eo + 1) * P, q0 : q0 + 512],
                            f_sb[:],
                        )

    nc.compile()
    return nc


def _re3(a):
    """[K, E] -> [P, K//P, E] host rearrange for contiguous weight DMAs."""
    return np.ascontiguousarray(a.reshape(-1, P, a.shape[1]).transpose(1, 0, 2))


def _host_inputs(x, w_qkv, w_o):
    """Build the 8 per-core input maps from full inputs."""
    x = np.asarray(x, dtype=np.float32)
    w_qkv = np.asarray(w_qkv, dtype=np.float32)
    w_o = np.asarray(w_o, dtype=np.float32)

    # rope tables, replicated on both 64-halves of the head dim
    half = HEAD_DIM // 2
    inv_freq = 1.0 / (
        THETA ** (np.arange(0, HEAD_DIM, 2, dtype=np.float32) / HEAD_DIM)
    )
    ang = np.arange(T, dtype=np.float32)[:, None] * inv_freq[None, :]  # [T, 64]
    cos = np.cos(ang).T.astype(np.float32)  # [64, T]
    sin = np.sin(ang).T.astype(np.float32)
    cos_t = np.ascontiguousarray(np.concatenate([cos, cos], axis=0))  # [128, T]
    sin_t = np.ascontiguousarray(np.concatenate([sin, sin], axis=0))

    ones_m = np.ones((P, P), dtype=np.float32)
    pswap = np.zeros((P, P), dtype=np.float32)
    for p in range(half):
        pswap[p, p + half] = 1.0    # out[m=p+64] += ys[p]
        pswap[p + half, p] = -1.0   # out[m=p]    -= ys[p+64]

    in_maps = []
    for c in range(N_CORES):
        b, h = c // 2, c % 2
        qrows = slice(h * EQ, (h + 1) * EQ)
        krows = slice(Q_DIM + h * EKV, Q_DIM + (h + 1) * EKV)
        vrows = slice(Q_DIM + KV_DIM + h * EKV, Q_DIM + (h + 1) * EKV + KV_DIM)
        wq_r = _re3(np.ascontiguousarray(w_qkv[qrows].T))     # [P, 16, 1024]
        wq_r4 = np.ascontiguousarray(
            wq_r.reshape(P, KSUB, H_Q, P).transpose(2, 0, 1, 3)
        )  # [H_Q, P, 16, 128]
        in_maps.append(
            {
                "xt": np.ascontiguousarray(x[b].T),
                "wq": wq_r4,
                "wk": _re3(np.ascontiguousarray(w_qkv[krows].T)),
                "wv": _re3(np.ascontiguousarray(w_qkv[vrows].T)),
                "wo": _re3(
                    np.ascontiguousarray(w_o[:, h * EQ : (h + 1) * EQ].T)
                ).reshape(P, H_Q, D),
                "cos_t": cos_t,
                "sin_t": sin_t,
                "ones_m": ones_m,
                "pswap": pswap,
            }
        )
    return in_maps


def _gather(results):
    out = np.empty((B, T, D), dtype=np.float32)
    for b in range(B):
        acc = results[2 * b]["out_t"] + results[2 * b + 1]["out_t"]
        out[b] = acc.T
    return out


_NC_CACHE = []


def _get_module():
    if not _NC_CACHE:
        _NC_CACHE.append(_build_module())
    return _NC_CACHE[0]


def kernel(x, w_qkv, w_o):
    nc = _get_module()
    in_maps = _host_inputs(x, w_qkv, w_o)
    results = bass2jax.run_bass_via_pjrt(nc, in_maps, n_cores=N_CORES)
    return _gather(results)



# revision 3
# speedup vs baseline: 1.3043x; 1.3043x over previous
"""Causal GQA attention block (QK L2-norm + RoPE) for 8 trn2 NeuronCores.

Sharding: tensor-parallel over head-halves (2) x data-parallel over batch (4).
Core c handles batch c//2 and heads [h*8, h*8+8) with h = c%2 (kv heads
[h*2, h*2+2)).  Each core computes its partial output-projection; the host
sums the two partials per batch and transposes back.

Numerics/dtype strategy (tolerance budget ~2e-2 L2):
  - Q/K projection runs in fp8e4m3 with MatmulPerfMode.DoubleRow
    (256-deep contraction per pass -> half the PE passes).  QK-norm makes
    scores insensitive to q/k quantization: the softmax sees scale*s with
    scale=0.088, so a ~5% q/k perturbation moves weights by ~6e-4.
  - Roped, normalized q/k are stored as fp8 in SBUF ([128, T] layout),
    so phase 2 needs no DRAM round-trip and QK^T runs as plain fp8
    matmuls (1 cyc/col at any width -> exact causal diagonal streaming).
  - V path, attention weights, and the output projection run in bf16
    (same PE rate as f32r but FWL weight loads remove LDWEIGHTS bubbles,
    and any-width streaming removes the f32r >=256-column restriction).
  - 1/||v|| uses the Abs_reciprocal_sqrt activation (norm-scalar errors
    are damped by scale*s, so table accuracy is irrelevant here).
Phase 1 is software-pipelined 2 deep so the PE never waits on the
ACT/DVE/Pool chain of the previous projection block.
"""

import numpy as np
import ml_dtypes

import concourse.mybir as mybir
import concourse.tile as tile
from concourse import bacc
from concourse import bass2jax

F32 = mybir.dt.float32
F32R = mybir.dt.float32r
BF16 = mybir.dt.bfloat16
FP8 = mybir.dt.float8e4
AF = mybir.ActivationFunctionType
DR = mybir.MatmulPerfMode.DoubleRow

P = 128
B, T, D = 4, 2048, 2048
N_HEADS, HEAD_DIM, N_KV = 16, 128, 4
Q_DIM = N_HEADS * HEAD_DIM          # 2048
KV_DIM = N_KV * HEAD_DIM            # 512
H_Q = 8                             # q heads per core
H_KV = 2                            # kv heads per core
EQ = H_Q * HEAD_DIM                 # 1024 q features per core
EKV = H_KV * HEAD_DIM               # 256
SCALE = 0.08838834764831845
THETA = 10000.0

KSUB = D // P                       # 16 contraction subtiles
KPAIR = KSUB // 2                   # 8 DoubleRow pairs
N_CORES = 8
NQ4 = T // 512                      # 4 512-token quarters
NTB = T // P                        # 16 128-token blocks
NBLK = H_KV + H_Q                   # 10 projection feature blocks (K first)


def _build_module():
    nc = bacc.Bacc("TRN2", target_bir_lowering=False, debug=False)

    x8 = nc.dram_tensor("x8", [P, KSUB, T], FP8, kind="ExternalInput")
    xb = nc.dram_tensor("xb", [P, KSUB, T], BF16, kind="ExternalInput")
    wqk8 = nc.dram_tensor("wqk8", [P, KPAIR, 2, NBLK * P], FP8,
                          kind="ExternalInput")
    wvb = nc.dram_tensor("wvb", [P, KSUB, EKV], BF16, kind="ExternalInput")
    wob = nc.dram_tensor("wob", [P, H_Q, D], BF16, kind="ExternalInput")
    cos_b = nc.dram_tensor("cos_b", [P, T], BF16, kind="ExternalInput")
    sin_b = nc.dram_tensor("sin_b", [P, T], BF16, kind="ExternalInput")
    ones_b = nc.dram_tensor("ones_b", [P, P], BF16, kind="ExternalInput")
    pswap_b = nc.dram_tensor("pswap_b", [P, P], BF16, kind="ExternalInput")
    out_t = nc.dram_tensor("out_t", [D, T], F32, kind="ExternalOutput")

    with tile.TileContext(nc) as tc:
        with (
            tc.tile_pool(name="persist", bufs=1) as persist,
            tc.tile_pool(name="kv_persist", bufs=1) as kvp,
            tc.tile_pool(name="att_sb", bufs=5) as att_sb,
        ):
            ones_sb = persist.tile([P, P], BF16)
            psw_sb = persist.tile([P, P], BF16)
            nc.sync.dma_start(ones_sb[:], ones_b.ap())
            nc.sync.dma_start(psw_sb[:], pswap_b.ap())
            # roped+normed K^T / Q^T in fp8, resident for phase 2
            k8_sb = kvp.tile([P, H_KV, T], FP8)
            q8_sb = kvp.tile([P, H_Q, T], FP8)
            v_sb = kvp.tile([P, NTB, EKV], BF16)

            # ---------------- phase 1: qkv proj + L2 norm + rope ----------
            with (
                tc.tile_pool(name="x8res", bufs=2) as x8res,
                tc.tile_pool(name="xbres", bufs=2) as xbres,
                tc.tile_pool(name="wres", bufs=1) as wres,
                tc.tile_pool(name="p1tmp", bufs=3) as p1tmp,
                tc.tile_pool(name="trig", bufs=1) as trig,
                tc.tile_pool(name="pp", bufs=2, space="PSUM") as pp,
                tc.tile_pool(name="pssq", bufs=2, space="PSUM") as pssq,
                tc.tile_pool(name="psw", bufs=2, space="PSUM") as psw,
                tc.tile_pool(name="pv", bufs=2, space="PSUM") as pv,
            ):
                wqk_sb = wres.tile([P, KPAIR, 2, NBLK * P], FP8)
                nc.sync.dma_start(wqk_sb[:], wqk8.ap())
                wv_sb = wres.tile([P, KSUB, EKV], BF16)
                nc.sync.dma_start(wv_sb[:], wvb.ap())
                cos_sb = trig.tile([P, T], BF16)
                sin_sb = trig.tile([P, T], BF16)
                nc.sync.dma_start(cos_sb[:], cos_b.ap())
                nc.sync.dma_start(sin_sb[:], sin_b.ap())

                # per-block pipeline state, keyed by flat block index
                st = {}

                def stage_dr(i, x8_q, tq):
                    """PE: 8 DoubleRow matmuls -> raw scores for block i."""
                    es = i % NBLK
                    raw = pp.tile([P, 512], F32, tag="raw")
                    w_sl = wqk_sb[:, :, :, es * P : (es + 1) * P]
                    for j in range(KPAIR):
                        nc.tensor.matmul(
                            raw[:],
                            w_sl[:, j],
                            x8_q[:, 2 * j : 2 * j + 2],
                            start=(j == 0),
                            stop=(j == KPAIR - 1),
                            perf_mode=DR,
                        )
                    st[i] = {"raw": raw, "es": es, "tq": tq}

                def stage_square(i):
                    s = st[i]
                    sq = p1tmp.tile([P, 512], BF16, tag="sq")
                    nc.scalar.activation(sq[:], s["raw"][:], AF.Square)
                    s["sq"] = sq

                def stage_ssq(i):
                    s = st[i]
                    ssq = pssq.tile([P, 512], F32, tag="ssq")
                    nc.tensor.matmul(ssq[:], ones_sb[:], s["sq"][:],
                                     start=True, stop=True)
                    s["ssq"] = ssq

                def stage_rsqrt(i):
                    s = st[i]
                    r = p1tmp.tile([P, 512], F32, tag="r")
                    nc.scalar.activation(r[:], s["ssq"][:],
                                         AF.Abs_reciprocal_sqrt)
                    s["r"] = r

                def stage_qn(i):
                    s = st[i]
                    qn = p1tmp.tile([P, 512], BF16, tag="qn")
                    nc.vector.tensor_mul(qn[:], s["raw"][:], s["r"][:])
                    s["qn"] = qn

                def stage_trig(i):
                    s = st[i]
                    tg = s["tq"] * 512
                    ys = p1tmp.tile([P, 512], BF16, tag="ys")
                    nc.gpsimd.tensor_mul(ys[:], s["qn"][:],
                                         sin_sb[:, tg : tg + 512])
                    qc = p1tmp.tile([P, 512], BF16, tag="qc")
                    nc.gpsimd.tensor_mul(qc[:], s["qn"][:],
                                         cos_sb[:, tg : tg + 512])
                    s["ys"] = ys
                    s["qc"] = qc

                def stage_swap(i):
                    s = st[i]
                    sw = psw.tile([P, 512], F32, tag="sw")
                    nc.tensor.matmul(sw[:], psw_sb[:], s["ys"][:],
                                     start=True, stop=True)
                    s["sw"] = sw

                def stage_out(i):
                    s = st[i]
                    es, tg = s["es"], s["tq"] * 512
                    if es < H_KV:
                        dest = k8_sb[:, es, tg : tg + 512]
                    else:
                        dest = q8_sb[:, es - H_KV, tg : tg + 512]
                    nc.vector.tensor_add(dest, s["sw"][:], s["qc"][:])
                    del st[i]

                def stage_v(tq, xb_q):
                    for tb in range(4):
                        tbg = tq * 4 + tb
                        v_ps = pv.tile([P, EKV], F32, tag="vp")
                        for ks in range(KSUB):
                            nc.tensor.matmul(
                                v_ps[:],
                                xb_q[:, ks, tb * P : (tb + 1) * P],
                                wv_sb[:, ks],
                                start=(ks == 0),
                                stop=(ks == KSUB - 1),
                            )
                        nc.scalar.copy(v_sb[:, tbg], v_ps[:])

                # pipelined emission: per block i emit PE work for i and
                # trailing elementwise work for i-1 / i-2
                for tq in range(NQ4):
                    t0 = tq * 512
                    x8_q = x8res.tile([P, KSUB, 512], FP8, tag="x8")
                    xb_q = xbres.tile([P, KSUB, 512], BF16, tag="xb")
                    for ks in range(KSUB):
                        nc.sync.dma_start(x8_q[:, ks],
                                          x8.ap()[:, ks, t0 : t0 + 512])
                    for ks in range(KSUB):
                        nc.sync.dma_start(xb_q[:, ks],
                                          xb.ap()[:, ks, t0 : t0 + 512])
                    for es in range(NBLK):
                        i = tq * NBLK + es
                        stage_dr(i, x8_q, tq)
                        if i >= 1:
                            stage_ssq(i - 1)
                        if i >= 2:
                            stage_swap(i - 2)
                        stage_square(i)
                        if i >= 1:
                            stage_rsqrt(i - 1)
                            stage_qn(i - 1)
                            stage_trig(i - 1)
                        if i >= 2:
                            stage_out(i - 2)
                    stage_v(tq, xb_q)
                # drain the pipeline (blocks 38, 39)
                last = NQ4 * NBLK - 1
                stage_ssq(last)
                stage_swap(last - 1)
                stage_rsqrt(last)
                stage_qn(last)
                stage_trig(last)
                stage_out(last - 1)
                stage_swap(last)
                stage_out(last)

            # ------- phase 2: attention + output projection per q-tile ----
            with (
                tc.tile_pool(name="wores", bufs=1) as wores,
                tc.tile_pool(name="p2tmp", bufs=2) as p2tmp,
                tc.tile_pool(name="oall", bufs=2) as oall,
                tc.tile_pool(name="fout", bufs=3) as fout,
                tc.tile_pool(name="psc", bufs=2, space="PSUM") as psc,
                tc.tile_pool(name="pav", bufs=1, space="PSUM") as pav,
                tc.tile_pool(name="psum2", bufs=1, space="PSUM") as psum2,
                tc.tile_pool(name="pf", bufs=2, space="PSUM") as pf,
            ):
                # w_o via the gpsimd DMA queue, split per slab, so it never
                # head-of-line-blocks the sync queue
                wo_sb = wores.tile([P, H_Q, D], BF16)
                for ei in range(H_Q):
                    nc.gpsimd.dma_start(wo_sb[:, ei], wob.ap()[:, ei])
                for qt in range(NQ4):
                    q0 = qt * 512
                    nkb = (qt + 1) * 4
                    o_all = oall.tile([P, H_Q, 512], BF16, tag="oa")
                    for hd in range(H_Q):
                        kvi = hd // 4
                        q_t = q8_sb[:, hd, q0 : q0 + 512]
                        atts = []

                        def diag_off(kb):
                            # fully-masked left columns of a diagonal block;
                            # fp8/bf16 matmuls run 1 cyc/col at any width so
                            # stream the exact causal remainder
                            off = kb * P - q0
                            return off if off in (P, 2 * P, 3 * P) else 0

                        for kb0 in range(0, nkb, 2):
                            npair = min(2, nkb - kb0)
                            sc_ps = psc.tile([P, 1024], F32, tag="sc")
                            att = att_sb.tile([P, 1024], BF16, tag="att")
                            for j in range(npair):
                                kb = kb0 + j
                                off = diag_off(kb)
                                nc.tensor.matmul(
                                    sc_ps[:, j * 512 + off : (j + 1) * 512],
                                    k8_sb[:, kvi, kb * P : (kb + 1) * P],
                                    q_t[:, off:],
                                    start=True,
                                    stop=True,
                                )
                            offs = [diag_off(kb0 + j) for j in range(npair)]
                            if not any(offs):
                                nc.scalar.activation(
                                    att[:, : npair * 512],
                                    sc_ps[:, : npair * 512],
                                    AF.Exp,
                                    scale=SCALE,
                                )
                            else:
                                for j in range(npair):
                                    sl = slice(j * 512 + offs[j], (j + 1) * 512)
                                    nc.scalar.activation(
                                        att[:, sl], sc_ps[:, sl], AF.Exp,
                                        scale=SCALE,
                                    )
                            for j in range(npair):
                                kb = kb0 + j
                                off = offs[j]
                                # zero future positions on diagonal blocks
                                if q0 < (kb + 1) * P and kb * P < q0 + 512:
                                    sl = slice(j * 512 + off, (j + 1) * 512)
                                    nc.gpsimd.affine_select(
                                        out=att[:, sl],
                                        in_=att[:, sl],
                                        compare_op=mybir.AluOpType.is_ge,
                                        fill=0.0,
                                        base=q0 + off - kb * P,
                                        pattern=[[1, 512 - off]],
                                        channel_multiplier=-1,
                                    )
                                atts.append((kb, att[:, j * 512 : (j + 1) * 512]))
                        o_ps = pav.tile([P, 512], F32, tag="av")
                        for kb, a_slice in atts:
                            off = diag_off(kb)
                            nc.tensor.matmul(
                                o_ps[:, off:],
                                v_sb[:, kb, kvi * HEAD_DIM : (kvi + 1) * HEAD_DIM],
                                a_slice[:, off:],
                                start=(kb == 0),
                                stop=(kb == nkb - 1),
                            )
                        s_ps = psum2.tile([P, 512], F32, tag="sum")
                        for kb, a_slice in atts:
                            off = diag_off(kb)
                            nc.tensor.matmul(
                                s_ps[:, off:],
                                ones_sb[:],
                                a_slice[:, off:],
                                start=(kb == 0),
                                stop=(kb == nkb - 1),
                            )
                        rs = p2tmp.tile([P, 512], F32, tag="rs")
                        nc.vector.reciprocal_approx_fast(rs[:], s_ps[:])
                        nc.vector.tensor_mul(o_all[:, hd], o_ps[:], rs[:])
                    for eo in range(D // P):
                        f_ps = pf.tile([P, 512], F32, tag="f")
                        for ei in range(H_Q):
                            nc.tensor.matmul(
                                f_ps[:],
                                wo_sb[:, ei, eo * P : (eo + 1) * P],
                                o_all[:, ei],
                                start=(ei == 0),
                                stop=(ei == H_Q - 1),
                            )
                        f_sb = fout.tile([P, 512], F32, tag="fo")
                        nc.scalar.copy(f_sb[:], f_ps[:])
                        nc.sync.dma_start(
                            out_t.ap()[eo * P : (eo + 1) * P, q0 : q0 + 512],
                            f_sb[:],
                        )

    nc.compile()
    return nc


def _host_inputs(x, w_qkv, w_o):
    """Build the 8 per-core input maps from full inputs."""
    x = np.asarray(x, dtype=np.float32)
    w_qkv = np.asarray(w_qkv, dtype=np.float32)
    w_o = np.asarray(w_o, dtype=np.float32)

    # rope tables, replicated on both 64-halves of the head dim
    half = HEAD_DIM // 2
    inv_freq = 1.0 / (
        THETA ** (np.arange(0, HEAD_DIM, 2, dtype=np.float32) / HEAD_DIM)
    )
    ang = np.arange(T, dtype=np.float32)[:, None] * inv_freq[None, :]  # [T, 64]
    cos = np.cos(ang).T.astype(np.float32)  # [64, T]
    sin = np.sin(ang).T.astype(np.float32)
    cos_b = np.concatenate([cos, cos], axis=0).astype(ml_dtypes.bfloat16)
    sin_b = np.concatenate([sin, sin], axis=0).astype(ml_dtypes.bfloat16)

    ones_b = np.ones((P, P), dtype=ml_dtypes.bfloat16)
    pswap = np.zeros((P, P), dtype=np.float32)
    for p in range(half):
        pswap[p, p + half] = 1.0    # out[m=p+64] += ys[p]
        pswap[p + half, p] = -1.0   # out[m=p]    -= ys[p+64]
    pswap_b = pswap.astype(ml_dtypes.bfloat16)

    in_maps = []
    for c in range(N_CORES):
        b, h = c // 2, c % 2
        xt = np.ascontiguousarray(x[b].T)                  # [D, T]
        xt3 = xt.reshape(KSUB, P, T).transpose(1, 0, 2)    # [P, KSUB, T]
        x8 = np.ascontiguousarray(xt3).astype(ml_dtypes.float8_e4m3)
        xbm = np.ascontiguousarray(xt3).astype(ml_dtypes.bfloat16)

        # K blocks first (rows 2048 + h*256 ..), then Q blocks
        krows = w_qkv[Q_DIM + h * EKV : Q_DIM + (h + 1) * EKV]   # [256, D]
        qrows = w_qkv[h * EQ : (h + 1) * EQ]                     # [1024, D]
        wqk = np.concatenate([krows, qrows], axis=0)             # [1280, D]
        # wqk8[p, j, i, blk*128+m] = wqk[blk*128+m, (2j+i)*128+p]
        w4 = wqk.T.reshape(KPAIR, 2, P, NBLK * P).transpose(2, 0, 1, 3)
        wqk8 = np.ascontiguousarray(w4).astype(ml_dtypes.float8_e4m3)

        vrows = w_qkv[Q_DIM + KV_DIM + h * EKV : Q_DIM + KV_DIM + (h + 1) * EKV]
        wv3 = vrows.T.reshape(KSUB, P, EKV).transpose(1, 0, 2)   # [P, 16, 256]
        wvb = np.ascontiguousarray(wv3).astype(ml_dtypes.bfloat16)

        wo_c = w_o[:, h * EQ : (h + 1) * EQ].T                   # [1024, D]
        wo3 = wo_c.reshape(H_Q, P, D).transpose(1, 0, 2)         # [P, 8, D]
        wob = np.ascontiguousarray(wo3).astype(ml_dtypes.bfloat16)

        in_maps.append(
            {
                "x8": x8,
                "xb": xbm,
                "wqk8": wqk8,
                "wvb": wvb,
                "wob": wob,
                "cos_b": cos_b,
                "sin_b": sin_b,
                "ones_b": ones_b,
                "pswap_b": pswap_b,
            }
        )
    return in_maps


def _gather(results):
    out = np.empty((B, T, D), dtype=np.float32)
    for b in range(B):
        acc = results[2 * b]["out_t"] + results[2 * b + 1]["out_t"]
        out[b] = acc.T
    return out


_NC_CACHE = []


def _get_module():
    if not _NC_CACHE:
        _NC_CACHE.append(_build_module())
    return _NC_CACHE[0]


def kernel(x, w_qkv, w_o):
    nc = _get_module()
    in_maps = _host_inputs(x, w_qkv, w_o)
    results = bass2jax.run_bass_via_pjrt(nc, in_maps, n_cores=N_CORES)
    return _gather(results)


# revision 10
# speedup vs baseline: 1.3370x; 1.0250x over previous
"""Causal GQA attention block (QK L2-norm + RoPE) for 8 trn2 NeuronCores.

Sharding: tensor-parallel over head-halves (2) x data-parallel over batch (4).
Core c handles batch c//2 and heads [h*8, h*8+8) with h = c%2 (kv heads
[h*2, h*2+2)).  Each core computes its partial output-projection; the host
sums the two partials per batch and transposes back.

Numerics/dtype strategy (tolerance budget ~2e-2 L2):
  - Q/K projection runs in fp8e4m3 with MatmulPerfMode.DoubleRow
    (256-deep contraction per pass -> half the PE passes).  QK-norm makes
    scores insensitive to q/k quantization: the softmax sees scale*s with
    scale=0.088, so a ~5% q/k perturbation moves weights by ~6e-4.
  - Roped, normalized q/k are stored as fp8 in SBUF ([128, T] layout),
    so phase 2 needs no DRAM round-trip and QK^T runs as plain fp8
    matmuls (1 cyc/col at any width -> exact causal diagonal streaming).
  - V path, attention weights, and the output projection run in bf16
    (same PE rate as f32r but FWL weight loads remove LDWEIGHTS bubbles,
    and any-width streaming removes the f32r >=256-column restriction).
  - 1/||v|| uses the Abs_reciprocal_sqrt activation (norm-scalar errors
    are damped by scale*s, so table accuracy is irrelevant here).
Phase 1 is software-pipelined 2 deep so the PE never waits on the
ACT/DVE/Pool chain of the previous projection block.
"""

import numpy as np
import ml_dtypes

import concourse.mybir as mybir
import concourse.tile as tile
from concourse import bacc
from concourse import bass2jax

F32 = mybir.dt.float32
F32R = mybir.dt.float32r
BF16 = mybir.dt.bfloat16
FP8 = mybir.dt.float8e4
AF = mybir.ActivationFunctionType
DR = mybir.MatmulPerfMode.DoubleRow

P = 128
B, T, D = 4, 2048, 2048
N_HEADS, HEAD_DIM, N_KV = 16, 128, 4
Q_DIM = N_HEADS * HEAD_DIM          # 2048
KV_DIM = N_KV * HEAD_DIM            # 512
H_Q = 8                             # q heads per core
H_KV = 2                            # kv heads per core
EQ = H_Q * HEAD_DIM                 # 1024 q features per core
EKV = H_KV * HEAD_DIM               # 256
SCALE = 0.08838834764831845
THETA = 10000.0

KSUB = D // P                       # 16 contraction subtiles
KPAIR = KSUB // 2                   # 8 DoubleRow pairs
N_CORES = 8
NQ4 = T // 512                      # 4 512-token quarters
NTB = T // P                        # 16 128-token blocks
NBLK = H_KV + H_Q                   # 10 projection feature blocks (K first)


def _build_module():
    nc = bacc.Bacc("TRN2", target_bir_lowering=False, debug=False)

    x8 = nc.dram_tensor("x8", [P, KSUB, T], FP8, kind="ExternalInput")
    xb = nc.dram_tensor("xb", [P, KSUB, T], BF16, kind="ExternalInput")
    wqk8 = nc.dram_tensor("wqk8", [P, NBLK, KPAIR, 2, P], FP8,
                          kind="ExternalInput")
    wvb = nc.dram_tensor("wvb", [P, KSUB, EKV], BF16, kind="ExternalInput")
    wob = nc.dram_tensor("wob", [P, H_Q, D], BF16, kind="ExternalInput")
    cos_b = nc.dram_tensor("cos_b", [P, T], BF16, kind="ExternalInput")
    sin_b = nc.dram_tensor("sin_b", [P, T], BF16, kind="ExternalInput")
    ones_b = nc.dram_tensor("ones_b", [P, P], BF16, kind="ExternalInput")
    pswap_b = nc.dram_tensor("pswap_b", [P, P], BF16, kind="ExternalInput")
    out_t = nc.dram_tensor("out_t", [D, T], F32, kind="ExternalOutput")

    with tile.TileContext(nc) as tc:
        with (
            tc.tile_pool(name="persist", bufs=1) as persist,
            tc.tile_pool(name="kv_persist", bufs=1) as kvp,
            tc.tile_pool(name="att_sb", bufs=18) as att_sb,
        ):
            ones_sb = persist.tile([P, P], BF16)
            psw_sb = persist.tile([P, P], BF16)
            nc.sync.dma_start(ones_sb[:], ones_b.ap())
            nc.sync.dma_start(psw_sb[:], pswap_b.ap())
            # roped+normed K^T / Q^T in fp8, resident for phase 2
            k8_sb = kvp.tile([P, H_KV, T], FP8)
            q8_sb = kvp.tile([P, H_Q, T], FP8)
            v_sb = kvp.tile([P, NTB, EKV], BF16)

            # ---------------- phase 1: qkv proj + L2 norm + rope ----------
            with (
                tc.tile_pool(name="x8res", bufs=2) as x8res,
                tc.tile_pool(name="xbres", bufs=2) as xbres,
                tc.tile_pool(name="wres", bufs=1) as wres,
                tc.tile_pool(name="p1tmp", bufs=3) as p1tmp,
                tc.tile_pool(name="trig", bufs=1) as trig,
                tc.tile_pool(name="pp", bufs=2, space="PSUM") as pp,
                tc.tile_pool(name="pssq", bufs=2, space="PSUM") as pssq,
                tc.tile_pool(name="psw", bufs=2, space="PSUM") as psw,
                tc.tile_pool(name="pv", bufs=2, space="PSUM") as pv,
            ):
                # critical-path-first DMA order: K-block weights, then the
                # first x8 quarter, then the rest; bulk tensors that are
                # needed later (xb/wv/trig) ride the gpsimd SWDGE queue so
                # they never head-of-line-block the first projections
                wqk_sb = wres.tile([P, NBLK, KPAIR, 2, P], FP8)
                wv_sb = wres.tile([P, KSUB, EKV], BF16)
                cos_sb = trig.tile([P, T], BF16)
                sin_sb = trig.tile([P, T], BF16)
                nc.sync.dma_start(wqk_sb[:, 0], wqk8.ap()[:, 0])
                nc.sync.dma_start(wqk_sb[:, 1], wqk8.ap()[:, 1])
                x8_q0 = x8res.tile([P, KSUB, 512], FP8, tag="x8")
                for ks in range(KSUB):
                    nc.sync.dma_start(x8_q0[:, ks], x8.ap()[:, ks, 0:512])
                for blk in range(2, NBLK):
                    nc.sync.dma_start(wqk_sb[:, blk], wqk8.ap()[:, blk])
                nc.gpsimd.dma_start(cos_sb[:], cos_b.ap())
                nc.gpsimd.dma_start(sin_sb[:], sin_b.ap())
                nc.gpsimd.dma_start(wv_sb[:], wvb.ap())
                xb_q0 = xbres.tile([P, KSUB, 512], BF16, tag="xb")
                for ks in range(KSUB):
                    nc.gpsimd.dma_start(xb_q0[:, ks], xb.ap()[:, ks, 0:512])

                # per-block pipeline state, keyed by flat block index
                st = {}

                def stage_dr(i, x8_q, tq):
                    """PE: 8 DoubleRow matmuls -> raw scores for block i."""
                    es = i % NBLK
                    raw = pp.tile([P, 512], F32, tag="raw")
                    for j in range(KPAIR):
                        nc.tensor.matmul(
                            raw[:],
                            wqk_sb[:, es, j],
                            x8_q[:, 2 * j : 2 * j + 2],
                            start=(j == 0),
                            stop=(j == KPAIR - 1),
                            perf_mode=DR,
                        )
                    st[i] = {"raw": raw, "es": es, "tq": tq}

                def stage_square(i):
                    s = st[i]
                    sq = p1tmp.tile([P, 512], BF16, tag="sq")
                    nc.scalar.activation(sq[:], s["raw"][:], AF.Square)
                    s["sq"] = sq

                def stage_ssq(i):
                    s = st[i]
                    ssq = pssq.tile([P, 512], F32, tag="ssq")
                    nc.tensor.matmul(ssq[:], ones_sb[:], s["sq"][:],
                                     start=True, stop=True)
                    s["ssq"] = ssq

                def stage_rsqrt(i):
                    s = st[i]
                    r = p1tmp.tile([P, 512], F32, tag="r")
                    nc.scalar.activation(r[:], s["ssq"][:],
                                         AF.Abs_reciprocal_sqrt)
                    s["r"] = r

                def stage_qn(i):
                    s = st[i]
                    qn = p1tmp.tile([P, 512], BF16, tag="qn")
                    nc.vector.tensor_mul(qn[:], s["raw"][:], s["r"][:])
                    s["qn"] = qn

                def stage_trig(i):
                    s = st[i]
                    tg = s["tq"] * 512
                    ys = p1tmp.tile([P, 512], BF16, tag="ys")
                    nc.gpsimd.tensor_mul(ys[:], s["qn"][:],
                                         sin_sb[:, tg : tg + 512])
                    qc = p1tmp.tile([P, 512], BF16, tag="qc")
                    nc.gpsimd.tensor_mul(qc[:], s["qn"][:],
                                         cos_sb[:, tg : tg + 512])
                    s["ys"] = ys
                    s["qc"] = qc

                def stage_swap(i):
                    s = st[i]
                    sw = psw.tile([P, 512], F32, tag="sw")
                    nc.tensor.matmul(sw[:], psw_sb[:], s["ys"][:],
                                     start=True, stop=True)
                    s["sw"] = sw

                def stage_out(i):
                    s = st[i]
                    es, tg = s["es"], s["tq"] * 512
                    if es < H_KV:
                        dest = k8_sb[:, es, tg : tg + 512]
                    else:
                        dest = q8_sb[:, es - H_KV, tg : tg + 512]
                    nc.vector.tensor_add(dest, s["sw"][:], s["qc"][:])
                    del st[i]

                def stage_v(tq, xb_q):
                    for tb in range(4):
                        tbg = tq * 4 + tb
                        v_ps = pv.tile([P, EKV], F32, tag="vp")
                        for ks in range(KSUB):
                            nc.tensor.matmul(
                                v_ps[:],
                                xb_q[:, ks, tb * P : (tb + 1) * P],
                                wv_sb[:, ks],
                                start=(ks == 0),
                                stop=(ks == KSUB - 1),
                            )
                        nc.scalar.copy(v_sb[:, tbg], v_ps[:])

                # pipelined emission: per block i emit PE work for i and
                # trailing elementwise work for i-1 / i-2
                for tq in range(NQ4):
                    t0 = tq * 512
                    if tq == 0:
                        x8_q, xb_q = x8_q0, xb_q0
                    else:
                        x8_q = x8res.tile([P, KSUB, 512], FP8, tag="x8")
                        xb_q = xbres.tile([P, KSUB, 512], BF16, tag="xb")
                        for ks in range(KSUB):
                            nc.sync.dma_start(x8_q[:, ks],
                                              x8.ap()[:, ks, t0 : t0 + 512])
                        for ks in range(KSUB):
                            nc.sync.dma_start(xb_q[:, ks],
                                              xb.ap()[:, ks, t0 : t0 + 512])
                    for es in range(NBLK):
                        i = tq * NBLK + es
                        stage_dr(i, x8_q, tq)
                        if i >= 1:
                            stage_ssq(i - 1)
                        if i >= 2:
                            stage_swap(i - 2)
                        stage_square(i)
                        if i >= 1:
                            stage_rsqrt(i - 1)
                            stage_qn(i - 1)
                            stage_trig(i - 1)
                        if i >= 2:
                            stage_out(i - 2)
                    stage_v(tq, xb_q)
                # drain the pipeline (blocks 38, 39)
                last = NQ4 * NBLK - 1
                stage_ssq(last)
                stage_swap(last - 1)
                stage_rsqrt(last)
                stage_qn(last)
                stage_trig(last)
                stage_out(last - 1)
                stage_swap(last)
                stage_out(last)

            # ------- phase 2: attention + output projection per q-tile ----
            with (
                tc.tile_pool(name="wores", bufs=1) as wores,
                tc.tile_pool(name="p2tmp", bufs=2) as p2tmp,
                tc.tile_pool(name="oall", bufs=2) as oall,
                tc.tile_pool(name="fout", bufs=3) as fout,
                tc.tile_pool(name="psc", bufs=2, space="PSUM") as psc,
                tc.tile_pool(name="pav", bufs=1, space="PSUM") as pav,
                tc.tile_pool(name="psum2", bufs=1, space="PSUM") as psum2,
                tc.tile_pool(name="pf", bufs=2, space="PSUM") as pf,
            ):
                # w_o via the gpsimd DMA queue, split per slab, so it never
                # head-of-line-blocks the sync queue
                wo_sb = wores.tile([P, H_Q, D], BF16)
                for ei in range(H_Q):
                    nc.gpsimd.dma_start(wo_sb[:, ei], wob.ap()[:, ei])
                for qt in range(NQ4):
                    q0 = qt * 512
                    nkb = (qt + 1) * 4

                    def diag_off(kb):
                        # fully-masked left columns of a diagonal block;
                        # fp8/bf16 matmuls run 1 cyc/col at any width so
                        # stream the exact causal remainder
                        off = kb * P - q0
                        return off if off in (P, 2 * P, 3 * P) else 0

                    def head_scores(hd):
                        """QK^T + exp + causal mask for one head."""
                        kvi = hd // 4
                        q_t = q8_sb[:, hd, q0 : q0 + 512]
                        atts = []
                        for kb0 in range(0, nkb, 2):
                            npair = min(2, nkb - kb0)
                            sc_ps = psc.tile([P, 1024], F32, tag="sc")
                            att = att_sb.tile([P, 1024], BF16, tag="att")
                            for j in range(npair):
                                kb = kb0 + j
                                off = diag_off(kb)
                                nc.tensor.matmul(
                                    sc_ps[:, j * 512 + off : (j + 1) * 512],
                                    k8_sb[:, kvi, kb * P : (kb + 1) * P],
                                    q_t[:, off:],
                                    start=True,
                                    stop=True,
                                )
                            offs = [diag_off(kb0 + j) for j in range(npair)]
                            if not any(offs):
                                nc.scalar.activation(
                                    att[:, : npair * 512],
                                    sc_ps[:, : npair * 512],
                                    AF.Exp,
                                    scale=SCALE,
                                )
                            else:
                                for j in range(npair):
                                    sl = slice(j * 512 + offs[j], (j + 1) * 512)
                                    nc.scalar.activation(
                                        att[:, sl], sc_ps[:, sl], AF.Exp,
                                        scale=SCALE,
                                    )
                            for j in range(npair):
                                kb = kb0 + j
                                off = offs[j]
                                # zero future positions on diagonal blocks
                                if q0 < (kb + 1) * P and kb * P < q0 + 512:
                                    sl = slice(j * 512 + off, (j + 1) * 512)
                                    nc.gpsimd.affine_select(
                                        out=att[:, sl],
                                        in_=att[:, sl],
                                        compare_op=mybir.AluOpType.is_ge,
                                        fill=0.0,
                                        base=q0 + off - kb * P,
                                        pattern=[[1, 512 - off]],
                                        channel_multiplier=-1,
                                    )
                                atts.append(
                                    (kb, att[:, j * 512 : (j + 1) * 512])
                                )
                        return atts

                    def head_reduce(hd, atts, o_all):
                        """AV + row sums + normalize for one head."""
                        kvi = hd // 4
                        o_ps = pav.tile([P, 512], F32, tag="av")
                        for kb, a_slice in atts:
                            off = diag_off(kb)
                            nc.tensor.matmul(
                                o_ps[:, off:],
                                v_sb[:, kb,
                                     kvi * HEAD_DIM : (kvi + 1) * HEAD_DIM],
                                a_slice[:, off:],
                                start=(kb == 0),
                                stop=(kb == nkb - 1),
                            )
                        s_ps = psum2.tile([P, 512], F32, tag="sum")
                        for kb, a_slice in atts:
                            off = diag_off(kb)
                            nc.tensor.matmul(
                                s_ps[:, off:],
                                ones_sb[:],
                                a_slice[:, off:],
                                start=(kb == 0),
                                stop=(kb == nkb - 1),
                            )
                        rs = p2tmp.tile([P, 512], F32, tag="rs")
                        nc.vector.reciprocal_approx_fast(rs[:], s_ps[:])
                        nc.vector.tensor_mul(o_all[:, hd], o_ps[:], rs[:])

                    # heads pipelined one deep: QK/exp of head h overlaps
                    # AV/SUM of head h-1, hiding the exp->AV latency
                    o_all = oall.tile([P, H_Q, 512], BF16, tag="oa")
                    prev = None
                    for hd in range(H_Q):
                        atts = head_scores(hd)
                        if prev is not None:
                            head_reduce(prev[0], prev[1], o_all)
                        prev = (hd, atts)
                    head_reduce(prev[0], prev[1], o_all)
                    for eo in range(D // P):
                        f_ps = pf.tile([P, 512], F32, tag="f")
                        for ei in range(H_Q):
                            nc.tensor.matmul(
                                f_ps[:],
                                wo_sb[:, ei, eo * P : (eo + 1) * P],
                                o_all[:, ei],
                                start=(ei == 0),
                                stop=(ei == H_Q - 1),
                            )
                        f_sb = fout.tile([P, 512], F32, tag="fo")
                        nc.vector.tensor_copy(f_sb[:], f_ps[:])
                        nc.sync.dma_start(
                            out_t.ap()[eo * P : (eo + 1) * P, q0 : q0 + 512],
                            f_sb[:],
                        )

    nc.compile()
    return nc


def _host_inputs(x, w_qkv, w_o):
    """Build the 8 per-core input maps from full inputs."""
    x = np.asarray(x, dtype=np.float32)
    w_qkv = np.asarray(w_qkv, dtype=np.float32)
    w_o = np.asarray(w_o, dtype=np.float32)

    # rope tables, replicated on both 64-halves of the head dim
    half = HEAD_DIM // 2
    inv_freq = 1.0 / (
        THETA ** (np.arange(0, HEAD_DIM, 2, dtype=np.float32) / HEAD_DIM)
    )
    ang = np.arange(T, dtype=np.float32)[:, None] * inv_freq[None, :]  # [T, 64]
    cos = np.cos(ang).T.astype(np.float32)  # [64, T]
    sin = np.sin(ang).T.astype(np.float32)
    cos_b = np.concatenate([cos, cos], axis=0).astype(ml_dtypes.bfloat16)
    sin_b = np.concatenate([sin, sin], axis=0).astype(ml_dtypes.bfloat16)

    ones_b = np.ones((P, P), dtype=ml_dtypes.bfloat16)
    pswap = np.zeros((P, P), dtype=np.float32)
    for p in range(half):
        pswap[p, p + half] = 1.0    # out[m=p+64] += ys[p]
        pswap[p + half, p] = -1.0   # out[m=p]    -= ys[p+64]
    pswap_b = pswap.astype(ml_dtypes.bfloat16)

    in_maps = []
    for c in range(N_CORES):
        b, h = c // 2, c % 2
        xt = np.ascontiguousarray(x[b].T)                  # [D, T]
        xt3 = xt.reshape(KSUB, P, T).transpose(1, 0, 2)    # [P, KSUB, T]
        x8 = np.ascontiguousarray(xt3).astype(ml_dtypes.float8_e4m3)
        xbm = np.ascontiguousarray(xt3).astype(ml_dtypes.bfloat16)

        # K blocks first (rows 2048 + h*256 ..), then Q blocks
        krows = w_qkv[Q_DIM + h * EKV : Q_DIM + (h + 1) * EKV]   # [256, D]
        qrows = w_qkv[h * EQ : (h + 1) * EQ]                     # [1024, D]
        wqk = np.concatenate([krows, qrows], axis=0)             # [1280, D]
        # wqk8[p, blk, j, i, m] = wqk[blk*128+m, (2j+i)*128+p]
        w5 = wqk.T.reshape(KPAIR, 2, P, NBLK, P).transpose(2, 3, 0, 1, 4)
        wqk8 = np.ascontiguousarray(w5).astype(ml_dtypes.float8_e4m3)

        vrows = w_qkv[Q_DIM + KV_DIM + h * EKV : Q_DIM + KV_DIM + (h + 1) * EKV]
        wv3 = vrows.T.reshape(KSUB, P, EKV).transpose(1, 0, 2)   # [P, 16, 256]
        wvb = np.ascontiguousarray(wv3).astype(ml_dtypes.bfloat16)

        wo_c = w_o[:, h * EQ : (h + 1) * EQ].T                   # [1024, D]
        wo3 = wo_c.reshape(H_Q, P, D).transpose(1, 0, 2)         # [P, 8, D]
        wob = np.ascontiguousarray(wo3).astype(ml_dtypes.bfloat16)

        in_maps.append(
            {
                "x8": x8,
                "xb": xbm,
                "wqk8": wqk8,
                "wvb": wvb,
                "wob": wob,
                "cos_b": cos_b,
                "sin_b": sin_b,
                "ones_b": ones_b,
                "pswap_b": pswap_b,
            }
        )
    return in_maps


def _gather(results):
    out = np.empty((B, T, D), dtype=np.float32)
    for b in range(B):
        acc = results[2 * b]["out_t"] + results[2 * b + 1]["out_t"]
        out[b] = acc.T
    return out


_NC_CACHE = []


def _get_module():
    if not _NC_CACHE:
        _NC_CACHE.append(_build_module())
    return _NC_CACHE[0]


def kernel(x, w_qkv, w_o):
    nc = _get_module()
    in_maps = _host_inputs(x, w_qkv, w_o)
    results = bass2jax.run_bass_via_pjrt(nc, in_maps, n_cores=N_CORES)
    return _gather(results)
